# revision 42
# baseline (speedup 1.0000x reference)
"""AdaptiveFeatureFusion Trainium2 kernel (8 NeuronCores, data-parallel).

Math rewrite: softmax over 2 logits -> sigmoid of the logit difference.
  delta[b] = sum_ij v[b,i] * (W0 - W1)[i,j] * s[b,j] + (b0 - b1)
           = rowsum((v @ W0 - v @ W1) * s) + (b0 - b1)
  a[b]     = sigmoid(delta[b])
  out[b,:] = a[b] * v[b,:] + (1 - a[b]) * s[b,:] = s + a*(v - s)

Sharding: batch dim (512) split across 8 cores (64 rows each); the
(2, 768*768) fc weight is replicated and streamed through each core
(the 4.7 MB weight read dominates: ~13 us/core at ~358 GB/s HBM).

Per-core: vT via PE transposes (cast to bf16), U0-U1 accumulated into
ONE PSUM bank via bf16 matmuls with +vT / -vT (h=0 at partitions 0:64,
h=1 at 64:128 via tile_position) so the dot product runs on all 128
DVE lanes; a tiny pair-sum matmul folds the two half-row partials, then
sigmoid on ScalarE and the fused output on VectorE.

Empirical notes from trace-driven tuning on this stack:
 - each dma_start costs ~0.65 us of sequencer issue time -> few, large,
   contiguous chunks, shrinking toward the end of the stream;
 - a chunk's completion semaphore fires ~2-3 us after its data's
   stream position (queue-drain skew), so the last pieces are j-halves
   of one i-tile, each feeding a single matmul;
 - fused DVE reduce ops (tensor_tensor_reduce, affine_mul_reduce,
   accum_out) are broken on this HW path; fp32 matmul is 4x slow;
   float32r returns zeros; gpsimd elementwise and collectives
   (~80 us floor for 8-core AllGather/AllToAll) are not viable.
"""

import os
import sys

for _p in ("/opt/trn_rl_repo", "/opt/pypackages"):
    if os.path.isdir(_p) and _p not in sys.path:
        sys.path.append(_p)

import numpy as np

B = 512
D = 768
NCORES = 8
BPC = B // NCORES  # 64 rows per core
NT = D // 128  # 6 i-tiles
NH = 2  # N halves of 384
WCHUNKS = 3  # DMA chunks per W row (2 i-tiles each)

_CACHE = {}


def _build():
    from concourse import bacc, mybir
    from concourse import tile

    f32 = mybir.dt.float32
    bf16 = mybir.dt.bfloat16
    AluOp = mybir.AluOpType
    Act = mybir.ActivationFunctionType

    nc = bacc.Bacc(None, target_bir_lowering=False)

    w_ext = nc.declare_dram_parameter("fc_w", [2, D * D], f32, isOutput=False)
    # packed: [:, 0:768] = v rows; [:, 768:1536] = s rows;
    # [:, 1536:1600] = identity(64); [:, 1600:1602] = fc_b broadcast
    PK = 2 * D + BPC + 2
    pk_ext = nc.declare_dram_parameter("packed", [BPC, PK], f32, isOutput=False)
    # aux2: pair-sum matrix M[p, b] = (p % 64 == b)
    aux2_ext = nc.declare_dram_parameter("aux2", [128, BPC], f32, isOutput=False)
    out_ext = nc.declare_dram_parameter("out", [BPC, D], f32, isOutput=True)

    NW = D // NH  # 384

    with tile.TileContext(nc) as tc:
        with (
            tc.tile_pool(name="sb", bufs=1) as sb,
            tc.tile_pool(name="ps", bufs=1, space="PSUM") as ps,
            tc.tile_pool(name="tps", bufs=2, space="PSUM") as tps,
        ):
            # --- W stream on sync. Per k one f32 landing tile (128, NT*768),
            # free dim (i_tile, j) i_tile-major; DMA'd in chunks, each
            # converted to bf16 on DVE as it lands. The first W chunk is
            # issued BEFORE the packed input (the packed data isn't needed
            # until the transposes ~4 us in), and the last i-tile is split
            # into j-halves so the final dependency is a quarter-size piece
            # feeding a single matmul per k. --------------------------------
            w_sb = []
            wb_sb = []
            for k in range(2):
                w_sb.append(sb.tile([128, NT * D], f32, tag=f"w{k}", name=f"w{k}"))
                wb_sb.append(
                    sb.tile([128, NT * D], bf16, tag=f"wb{k}", name=f"wb{k}")
                )
            pk_sb = sb.tile([BPC, PK], f32, tag="pk")
            v_sb = pk_sb[:, 0:D]
            s_sb = pk_sb[:, D : 2 * D]
            aux_sb = pk_sb[:, 2 * D : PK]
            aux2_sb = sb.tile([128, BPC], f32, tag="aux2")
            # s packed two-column-halves-per-partition-pair, built on-chip
            # from the packed input via SBUF->SBUF DMAs (fabric, not HBM)
            s2_sb = sb.tile([128, NW], f32, tag="s2")

            # (k, t0, tpc, h) pieces in stream order; h=None -> full-width.
            # One big 3-tile chunk per k, then single i-tiles, then j-halves
            # of the last i-tile: completion semaphores (and their casts)
            # arrive at ever finer granularity toward the end of the stream.
            chunks = []
            for tpc, t0 in [(3, 0), (1, 3), (1, 4)]:
                for k in range(2):
                    chunks.append((k, t0, tpc, None))
            for h in range(NH):
                for k in range(2):
                    chunks.append((k, NT - 1, 1, h))

            for ci, (k, t0, tpc, h) in enumerate(chunks):
                if h is None:
                    src = w_ext[k, t0 * 128 * D : (t0 + tpc) * 128 * D]
                    src = src.rearrange("(t p j) -> p t j", t=tpc, p=128, j=D)
                    sl = slice(t0 * D, (t0 + tpc) * D)
                    dst = w_sb[k][:, sl].rearrange("p (t j) -> p t j", t=tpc, j=D)
                else:
                    # j-half of i-tile t0: per-partition 384 floats, 3072 B
                    # stride in DRAM
                    src = w_ext[k, t0 * 128 * D : (t0 + 1) * 128 * D]
                    src = src.rearrange("(p j) -> p j", p=128, j=D)
                    src = src[:, h * NW : (h + 1) * NW]
                    sl = slice(t0 * D + h * NW, t0 * D + (h + 1) * NW)
                    dst = w_sb[k][:, sl]
                nc.sync.dma_start(out=dst, in_=src)
                nc.vector.tensor_copy(wb_sb[k][:, sl], w_sb[k][:, sl])
                if ci == 0:
                    nc.sync.dma_start(out=pk_sb[:, :], in_=pk_ext[:, :])
                if ci == 1:
                    nc.scalar.dma_start(out=aux2_sb[:, :], in_=aux2_ext[:, :])
                    nc.scalar.dma_start(
                        out=s2_sb[0:BPC, :], in_=pk_sb[:, D : D + NW]
                    )
                    nc.scalar.dma_start(
                        out=s2_sb[BPC : 2 * BPC, :], in_=pk_sb[:, D + NW : 2 * D]
                    )

            # --- vT via PE transposes, cast to bf16 in the PSUM->SBUF copy;
            # negated copy feeds the k=1 matmuls so U0 - U1 accumulates
            # directly in PSUM. ---------------------------------------------
            vt_p = sb.tile([128, NT * BPC], bf16, tag="vtp")
            vt_n = sb.tile([128, NT * BPC], bf16, tag="vtn")
            for t in range(NT):
                tp = tps.tile([128, BPC], f32, tag="tp")
                nc.tensor.transpose(
                    tp[:, :], v_sb[:, t * 128 : (t + 1) * 128], aux_sb[:, 0:BPC]
                )
                nc.vector.tensor_copy(vt_p[:, t * BPC : (t + 1) * BPC], tp[:, :])
                nc.vector.tensor_scalar_mul(
                    vt_n[:, t * BPC : (t + 1) * BPC], tp[:, :], -1.0
                )
            vt_k = [vt_p, vt_n]

            # --- bias difference (per-partition, from broadcast aux cols) --
            bd_bc = sb.tile([BPC, 1], f32, tag="bdbc")
            nc.vector.tensor_sub(
                bd_bc[:, :], aux_sb[:, BPC : BPC + 1], aux_sb[:, BPC + 1 : BPC + 2]
            )

            # --- vms = v - s (early; only needs v and s) -------------------
            vms_sb = sb.tile([BPC, D], f32, tag="vms")
            nc.vector.tensor_sub(vms_sb[:, :], v_sb[:, :], s_sb[:, :])

            # --- U0 - U1 = v @ W0 + (-v) @ W1, accumulated in ONE PSUM bank:
            # h=0 writes partitions 0:64, h=1 writes 64:128 (tile_position
            # selects the PE column group), so the dot product below runs on
            # all 128 DVE lanes. --------------------------------------------
            u_ps = ps.tile([2 * BPC, NW], f32, tag="u")
            # Accumulate in stream-arrival order (adds commute in PSUM).
            mm_order = []  # (k, t, h)
            for k, t0, tpc, h in chunks:
                for t in range(t0, t0 + tpc):
                    hs = range(NH) if h is None else (h,)
                    for hh in hs:
                        mm_order.append((k, t, hh))
            first_h = {0: None, 1: None}
            last_h = {}
            for i, (k, t, h) in enumerate(mm_order):
                if first_h[h] is None:
                    first_h[h] = i
                last_h[h] = i
            for i, (k, t, h) in enumerate(mm_order):
                nc.tensor.matmul(
                    u_ps[h * BPC : (h + 1) * BPC, :],
                    vt_k[k][:, t * BPC : (t + 1) * BPC],
                    wb_sb[k][:, t * D + h * NW : t * D + (h + 1) * NW],
                    start=(i == first_h[h]),
                    stop=(i == last_h[h]),
                    tile_position=(0, h * BPC),
                    skip_group_check=True,
                )

            # --- delta = rowsum((U0-U1) * s), on 128 lanes -----------------
            # (tensor_tensor_reduce crashes TRN2 HW via this stack)
            scr_sb = sb.tile([2 * BPC, NW], f32, tag="scr")
            dpk_sb = sb.tile([2 * BPC, 1], f32, tag="dpk")
            nc.vector.tensor_mul(scr_sb[:, :], u_ps[:, :], s2_sb[:, :])
            nc.vector.reduce_sum(dpk_sb[:, :], scr_sb[:, :], mybir.AxisListType.X)
            # pair-sum the two half-row partials: delta = M^T @ dpk
            d_ps = ps.tile([BPC, 1], f32, tag="dps")
            nc.tensor.matmul(d_ps[:, :], aux2_sb[:, :], dpk_sb[:, :])

            # --- a = sigmoid(delta + (b0-b1)) ------------------------------
            a_sb = sb.tile([BPC, 1], f32, tag="a")
            nc.scalar.activation(
                a_sb[:, :], d_ps[:, :], Act.Sigmoid, bias=bd_bc[:, :], scale=1.0
            )

            # --- out = s + a*(v-s) -----------------------------------------
            o_sb = sb.tile([BPC, D], f32, tag="o")
            nc.vector.scalar_tensor_tensor(
                o_sb[:, :],
                vms_sb[:, :],
                a_sb[:, :],
                s_sb[:, :],
                AluOp.mult,
                AluOp.add,
            )
            nc.scalar.dma_start(out=out_ext[:, :], in_=o_sb[:, :])

    nc.compile()
    return nc


def make_in_maps(v_x, s_x, fc_w, fc_b):
    v_x = np.ascontiguousarray(v_x, dtype=np.float32)
    s_x = np.ascontiguousarray(s_x, dtype=np.float32)
    fc_w = np.ascontiguousarray(fc_w, dtype=np.float32)
    fc_b = np.ascontiguousarray(fc_b, dtype=np.float32)

    PK = 2 * D + BPC + 2
    in_maps = []
    for m in range(NCORES):
        rows = slice(m * BPC, (m + 1) * BPC)
        packed = np.zeros((BPC, PK), dtype=np.float32)
        packed[:, 0:D] = v_x[rows]
        packed[:, D : 2 * D] = s_x[rows]
        packed[:, 2 * D : 2 * D + BPC] = np.eye(BPC, dtype=np.float32)
        packed[:, 2 * D + BPC :] = fc_b[None, :]
        aux2 = np.tile(np.eye(BPC, dtype=np.float32), (2, 1))
        in_maps.append({"fc_w": fc_w, "packed": packed, "aux2": aux2})
    return in_maps


def kernel(v_x, s_x, fc_w, fc_b):
    from concourse.bass_utils import run_bass_kernel_spmd

    key = "nc"
    if key not in _CACHE:
        _CACHE[key] = _build()
    nc = _CACHE[key]

    in_maps = make_in_maps(v_x, s_x, fc_w, fc_b)
    res = run_bass_kernel_spmd(nc, in_maps, core_ids=list(range(NCORES)))
    out = np.concatenate([res.results[m]["out"] for m in range(NCORES)], axis=0)
    return out.astype(np.float32)


if __name__ == "__main__":
    rng = np.random.default_rng(0)
    v = rng.standard_normal((B, D), dtype=np.float32)
    s = rng.standard_normal((B, D), dtype=np.float32)
    w = (rng.standard_normal((2, D * D), dtype=np.float32) * 0.01).astype(np.float32)
    b = np.zeros((2,), dtype=np.float32)
    o = kernel(v_x=v, s_x=s, fc_w=w, fc_b=b)
    print(o.shape, o.dtype)
